# revision 20
# baseline (speedup 1.0000x reference)
"""Trainium2 Bass kernel for CustomScaledDotProductAttention.

Full module: y = out_proj(softmax(q k^T / sqrt(D)) v) with fused qkv proj.
Shapes: x [2, 2048, 1024], H=16 heads, D=64.

Sharding (8 cores): core = b*4 + g, b = batch (2), g = head-group (4 heads).
Each core computes its batch's qkv projection restricted to its 4 heads,
attention for those heads, and the out-proj partial product (rows of
w_out.T owned by its heads).  Host sums the 4 partials per batch and adds
b_out (standard row-sharded tensor-parallel gather).

Device-side design notes:
  - All matmul operands are bf16 (1 cycle/row at any free-dim size, half
    the SBUF/DMA traffic and PE toggle power of f32).  Inputs are
    converted to bf16 on the host; PSUM accumulates f32.
  - qT/kT stored [d, n] (head-pairs packed in 128 partitions) so scores are
    computed TRANSPOSED (sT[m, n] = k q^T) -> softmax needs no transposes.
  - The two heads of a pair write one 2-bank PSUM tile ([128, 1024]); a
    SINGLE ScalarE exp instruction (fused *0.125) covers both banks,
    halving activation instruction overhead.  ScalarE is the phase-2
    pacing engine (~1.05us per m-step), so its overhead is the roofline.
  - V stored [m, d] with an all-ones 65th column per head; the attn@v
    matmul (M=65) then yields o^T[d, n] AND the softmax denominator l[n]
    (row 64) in a single pass of e^T through the PE.
  - attn@v emission LAGS the score/exp emission by 2 m-steps: the PE's
    in-order queue then never waits on ScalarE's exp, keeping the PE
    gapless so the HAM clock governor holds it at full p-state (the
    baseline spent all of phase 2 HAM-throttled to half clock).
  - Softmax normalization is off the PE critical path: o^T is copied out
    of PSUM by DVE (freeing the accumulator bank), then reciprocal /
    broadcast-DMA / multiply trail behind.
"""

import numpy as np
import ml_dtypes

import concourse.bass as bass
import concourse.mybir as mybir
import concourse.tile as tile
from concourse import library_config
from concourse.bass_utils import run_bass_kernel_spmd

F32 = mybir.dt.float32
BF16 = mybir.dt.bfloat16

B, N, C, H, D = 2, 2048, 1024, 16, 64
SCALE = D ** -0.5          # 0.125
HPC = 4                    # heads per core
N_CORES = 8
NK = C // 128              # 8 contraction chunks of 128
NM = N // 128              # 16 m-chunks
NN = N // 512              # 4 n-chunks of 512
VW = HPC * (D + 1)         # 260: v columns + ones column per head
LAG = 2                    # attn@v emission lag (m-steps) behind scores


def _emit(tc, nc, xT, wqk, bq, wv, wo, y, lbounce):
    PS = bass.MemorySpace.PSUM

    with (
        nc.allow_low_precision(reason="bf16 matmul operands; psum f32"),
        tc.tile_pool(name="persist", bufs=1) as pp,
        tc.tile_pool(name="qk", bufs=1) as qkp,
        tc.tile_pool(name="vp", bufs=1) as vp,
        tc.tile_pool(name="at", bufs=1) as atp,
    ):
        # ---- persistent tiles ----
        F32R = mybir.dt.float32r
        QK = [qkp.tile([128, N], BF16, tag=f"qk{j}", name=f"qk{j}")
              for j in range(4)]
        Vb = vp.tile([128, NM, VW], BF16, tag="vb")    # V[m] = Vb[:, m, :]
        # AT/wo stay f32r: DVE's bf16-out tensor_tensor is ~5x slower
        # (3.3us vs 0.7us per [64,512] mul), and f32r matmuls at N=512
        # stream at the same 1 cycle/row as bf16.
        AT = [atp.tile([128, N], F32R, tag=f"at{p}", name=f"at{p}")
              for p in range(2)]
        wo0 = pp.tile([128, 1024], F32R, tag="wo0")
        wo1 = pp.tile([128, 1024], F32R, tag="wo1")
        bqs = pp.tile([128, 4], F32, tag="bqs")
        nc.sync.dma_start(out=bqs, in_=bq[:, :])

        # ================= phase 1: projections =================
        with (
            tc.tile_pool(name="xw", bufs=1) as xw,
            tc.tile_pool(name="pps", bufs=8, space=PS) as pps,
        ):
            # chunked loads so matmuls can start before the full load lands
            xt = xw.tile([128, NK, N], BF16, tag="xt")
            wq = xw.tile([128, NK, 512], BF16, tag="wq")
            wvt = xw.tile([128, NK, VW], BF16, tag="wvt")
            for c in range(NK):
                nc.sync.dma_start(out=xt[:, c, :],
                                  in_=xT[c * 128:(c + 1) * 128, :])
                nc.sync.dma_start(out=wq[:, c, :],
                                  in_=wqk[c * 128:(c + 1) * 128, :])
                nc.sync.dma_start(out=wvt[:, c, :],
                                  in_=wv[c * 128:(c + 1) * 128, :])
            xt1f = xw.tile([1, N], F32, tag="xt1f")
            nc.vector.memset(xt1f, 1.0)
            xt1 = xw.tile([1, N], BF16, tag="xt1")
            nc.vector.tensor_copy(xt1, xt1f)
            wvb = xw.tile([1, VW], BF16, tag="wvb")
            nc.gpsimd.dma_start(out=wvb, in_=wv[C:C + 1, :])
            # wo loads early; consumed only in phase 3
            nc.sync.dma_start(out=wo0, in_=wo[0:128, :])
            nc.sync.dma_start(out=wo1, in_=wo[128:256, :])

            # qk-proj, contraction-chunk OUTER in two j-groups of 8 PSUM
            # chains each: the first matmuls only need DMA chunk 0, so
            # compute starts ~2us into the load instead of ~30us.
            # bias added by DVE during the PSUM->SBUF copy.
            for jg in range(2):
                ps = [[pps.tile([128, 512], F32, tag="pq", name="pq")
                       for n in range(NN)] for j in range(2)]
                for c in range(NK):
                    for j2 in range(2):
                        j = 2 * jg + j2
                        for n in range(NN):
                            nc.tensor.matmul(
                                ps[j2][n], wq[:, c, j * 128:(j + 1) * 128],
                                xt[:, c, n * 512:(n + 1) * 512],
                                start=(c == 0), stop=(c == NK - 1))
                for j2 in range(2):
                    j = 2 * jg + j2
                    for n in range(NN):
                        nc.vector.tensor_scalar_add(
                            QK[j][:, n * 512:(n + 1) * 512], ps[j2][n],
                            bqs[:, j:j + 1])

            # v-proj: V[m] = sum_c xt[c][:, m].T @ wvt[c] (+bias row)
            for m in range(NM):
                ps = pps.tile([128, VW], F32, tag="pq", name="pv")
                for c in range(NK):
                    nc.tensor.matmul(
                        ps, xt[:, c, m * 128:(m + 1) * 128], wvt[:, c, :],
                        start=(c == 0), stop=False)
                nc.tensor.matmul(
                    ps, xt1[:, m * 128:(m + 1) * 128], wvb,
                    start=False, stop=True)
                nc.scalar.copy(Vb[:, m, :], ps)

        # ========= phase 2: attention (+ out-proj on p=1) =========
        # otp's 4 banks are shared by the attn@v accumulators and the
        # out-proj accumulators: out-proj for n-chunk n is emitted right
        # after its normalization, so its matmuls fill the PE slack while
        # ScalarE works on the next n-chunk's exps, and y streams out of
        # PSUM over DMA with no SBUF staging.
        with (
            tc.tile_pool(name="sc", bufs=2, space=PS) as scp,
            tc.tile_pool(name="ot", bufs=2, space=PS) as otp,
            tc.tile_pool(name="yp", bufs=2, space=PS) as ypp,
            tc.tile_pool(name="et", bufs=3) as etp,
            tc.tile_pool(name="lv", bufs=4) as lvp,
            tc.tile_pool(name="orw", bufs=4) as orp,
            tc.tile_pool(name="bcs", bufs=4) as bcp,
            tc.tile_pool(name="ysb", bufs=3) as ysbp,
        ):
            def emit_outproj_mm(t, yp, k):
                # k-th matmul (of 4) of out-proj token chunk t
                ic, oc_ = divmod(k, 2)
                a, w = ((AT[0], wo0), (AT[1], wo1))[ic]
                nc.tensor.matmul(
                    yp[oc_], a[:, t * 128:(t + 1) * 128],
                    w[:, oc_ * 512:(oc_ + 1) * 512],
                    start=(ic == 0), stop=(ic == 1))

            def emit_outproj_store(t, yp):
                ysb = ysbp.tile([128, 1024], F32, tag="ysb")
                nc.vector.tensor_copy(ysb[:, 0:512], yp[0])
                nc.vector.tensor_copy(ysb[:, 512:1024], yp[1])
                nc.sync.dma_start(
                    out=y[t * 128:(t + 1) * 128, :], in_=ysb)

            op_ready = []   # out-proj chunks drippable (2 n-chunks old)
            op_fresh = []   # out-proj chunks from the last n-chunk
            mul_pend = []   # deferred normalization multiplies

            def emit_muls():
                for pm, nm, orw, bc, hh in mul_pend:
                    nc.vector.tensor_mul(
                        AT[pm][hh * 64:(hh + 1) * 64,
                               nm * 512:(nm + 1) * 512],
                        orw[0:64, :], bc)
                mul_pend.clear()

            for p in range(2):            # head pairs
                Qt, Kt = QK[2 * p], QK[2 * p + 1]
                for n in range(NN):
                    ot = [otp.tile([128, 512], F32, tag="ot", name="ot")
                          for _ in range(2)]

                    def emit_av(m, e):
                        for hh in range(2):
                            hcol = (2 * p + hh) * (D + 1)
                            nc.tensor.matmul(
                                ot[hh][0:65, :],
                                Vb[:, m, hcol:hcol + 65],
                                e[:, hh * 512:(hh + 1) * 512],
                                start=(m == 0), stop=(m == NM - 1))

                    pend = []
                    for m in range(NM):
                        # both heads' scores into one 2-bank PSUM tile
                        sp = scp.tile([128, 1024], F32, tag="sc")
                        nc.tensor.matmul(
                            sp[:, 0:512], Kt[0:64, m * 128:(m + 1) * 128],
                            Qt[0:64, n * 512:(n + 1) * 512],
                            start=True, stop=True, tile_position=(0, 0))
                        nc.tensor.matmul(
                            sp[:, 512:1024],
                            Kt[64:128, m * 128:(m + 1) * 128],
                            Qt[64:128, n * 512:(n + 1) * 512],
                            start=True, stop=True, tile_position=(64, 0))
                        # single exp over both banks; bf16 out
                        e = etp.tile([128, 1024], BF16, tag="et")
                        nc.scalar.activation(
                            e, sp, mybir.ActivationFunctionType.Exp,
                            scale=SCALE)
                        pend.append((m, e))
                        if len(pend) > LAG:
                            emit_av(*pend.pop(0))
                        if m == 7:
                            # drain the previous n-chunk's normalization
                            # muls here: their bounce-DMA broadcasts have
                            # had ~7 m-steps (7us) to land, so the DVE
                            # never blocks on them (a blocked DVE queue
                            # stalls the next ot evacuation -> PE -> HAM
                            # throttle).
                            emit_muls()
                        # drip one out-proj matmul per m-step for chunks
                        # TWO n-chunks back: fills PE slack without
                        # bursts, with normalization long complete.
                        if op_ready:
                            t, yp, k = op_ready[0]
                            emit_outproj_mm(t, yp, k)
                            if k == 3:
                                emit_outproj_store(t, yp)
                                op_ready.pop(0)
                            else:
                                op_ready[0] = (t, yp, k + 1)
                    for me in pend:
                        emit_av(*me)

                    # evacuate BOTH ot banks first (frees the 2-buf ring
                    # for the next n-chunk), launch the reciprocal +
                    # SBUF->SBUF broadcast DMA, and DEFER the multiplies
                    # to the next n-chunk so their latency never blocks
                    # the DVE queue.
                    orws = []
                    for hh in range(2):
                        orw = orp.tile([65, 512], F32, tag="orw",
                                       name="orw")
                        nc.vector.tensor_copy(orw, ot[hh][0:65, :])
                        orws.append(orw)
                    for hh in range(2):
                        orw = orws[hh]
                        linv = lvp.tile([1, 512], F32, tag="lv")
                        nc.vector.reciprocal(linv, orw[64:65, :])
                        idx = (p * NN + n) * 2 + hh
                        nc.gpsimd.dma_start(
                            out=lbounce[idx:idx + 1, :], in_=linv)
                        bc = bcp.tile([64, 512], F32, tag="bc", name="bc")
                        nc.gpsimd.dma_start(
                            out=bc,
                            in_=lbounce[idx:idx + 1, :]
                            .to_broadcast((64, 512)))
                        mul_pend.append((p, n, orw, bc, hh))

                    if p == 1:
                        # queue this n-chunk's out-proj; dripped one
                        # matmul per m-step two n-chunks later
                        op_ready.extend(op_fresh)
                        op_fresh.clear()
                        for t in range(4 * n, 4 * n + 4):
                            yp = [ypp.tile([128, 512], F32, tag="yp",
                                           name="yp") for _ in range(2)]
                            op_fresh.append((t, yp, 0))

            # drain deferred muls and remaining out-proj chunks
            emit_muls()
            for t, yp, k in op_ready + op_fresh:
                for kk in range(k, 4):
                    emit_outproj_mm(t, yp, kk)
                emit_outproj_store(t, yp)


def _split_multi_waits(nc):
    """Hoist all-but-one sem wait from instructions onto standalone
    EventSemaphore instructions: most TRN2 instruction encodings carry a
    single sync-wait slot (walrus: "Too many sync wait commands")."""
    import bass_rust
    nop_id = [0]
    for fn in nc.m.functions:
        for blk in fn.blocks:
            insts = blk.instructions
            out = []
            changed = False
            for ins in insts:
                si = ins.sync_info
                is_evsem = isinstance(ins, mybir.InstEventSemaphore)
                if (si is not None and si.on_wait is not None
                        and len(si.on_wait) > 1 and not is_evsem):
                    waits = list(si.on_wait)
                    for w in waits[:-1]:
                        ev = mybir.InstEventSemaphore(
                            name=f"waitev_{nop_id[0]}", engine=ins.engine)
                        nop_id[0] += 1
                        ev.sync_info = bass_rust.SyncInfo(
                            on_wait=[w], on_update=[])
                        out.append(ev)
                    ins.sync_info = bass_rust.SyncInfo(
                        on_wait=[waits[-1]],
                        on_update=list(si.on_update or []))
                    changed = True
                out.append(ins)
            if changed:
                blk.instructions = out


def build_bass(split_waits=True):
    nc = bass.Bass()
    xT = nc.dram_tensor("xT", [C, N], BF16, kind="ExternalInput")
    wqk = nc.dram_tensor("wqk", [C, 512], BF16, kind="ExternalInput")
    bq = nc.dram_tensor("bq", [128, 4], F32, kind="ExternalInput")
    wv = nc.dram_tensor("wv", [C + 1, VW], BF16, kind="ExternalInput")
    wo = nc.dram_tensor("wo", [2 * 128, 1024], mybir.dt.float32r,
                        kind="ExternalInput")
    y = nc.dram_tensor("y", [N, C], F32, kind="ExternalOutput")
    lbounce = nc.dram_tensor("lbounce", [16, 512], F32)
    with tile.TileContext(nc) as tc:
        _emit(tc, nc, xT, wqk, bq, wv, wo, y, lbounce)
    if split_waits:
        _split_multi_waits(nc)
    return nc


def prep_core_inputs(x, w_qkv, b_qkv, w_out, core):
    """Build the per-core input arrays (bf16 except the f32 qk bias)."""
    b, g = divmod(core, HPC)
    heads = [HPC * g + i for i in range(HPC)]
    bf = ml_dtypes.bfloat16

    xTa = np.ascontiguousarray(x[b].T.astype(bf))

    def q_rows(h):
        return w_qkv[h * D:(h + 1) * D]

    def k_rows(h):
        return w_qkv[C + h * D:C + (h + 1) * D]

    def v_rows(h):
        return w_qkv[2 * C + h * D:2 * C + (h + 1) * D]

    h0, h1, h2, h3 = heads
    wqk_rows = np.concatenate([
        q_rows(h0), q_rows(h1), k_rows(h0), k_rows(h1),
        q_rows(h2), q_rows(h3), k_rows(h2), k_rows(h3)], 0)   # [512, C]
    bqk = np.concatenate([
        b_qkv[h0 * D:(h0 + 1) * D], b_qkv[h1 * D:(h1 + 1) * D],
        b_qkv[C + h0 * D:C + (h0 + 1) * D],
        b_qkv[C + h1 * D:C + (h1 + 1) * D],
        b_qkv[h2 * D:(h2 + 1) * D], b_qkv[h3 * D:(h3 + 1) * D],
        b_qkv[C + h2 * D:C + (h2 + 1) * D],
        b_qkv[C + h3 * D:C + (h3 + 1) * D]], 0)               # [512]

    wv_aug = np.zeros((C + 1, VW), np.float32)
    for i, h in enumerate(heads):
        wv_aug[:C, i * (D + 1):i * (D + 1) + D] = v_rows(h).T
        wv_aug[C, i * (D + 1):i * (D + 1) + D] = \
            b_qkv[2 * C + h * D:2 * C + (h + 1) * D]
        wv_aug[C, i * (D + 1) + D] = 1.0

    woa = np.concatenate([w_out[:, h * D:(h + 1) * D].T for h in heads], 0)

    return {
        "xT": xTa,
        "wqk": np.ascontiguousarray(wqk_rows.T.astype(bf)),
        "bq": np.ascontiguousarray(
            bqk.reshape(4, 128).T.astype(np.float32)),
        "wv": np.ascontiguousarray(wv_aug.astype(bf)),
        "wo": np.ascontiguousarray(woa.astype(np.float32)),
    }


def assemble_output(partials, b_out):
    """partials: list of 8 [N, C] arrays (core order). Returns [B, N, C]."""
    y = np.empty((B, N, C), np.float32)
    for b in range(B):
        acc = partials[HPC * b].astype(np.float32).copy()
        for g in range(1, HPC):
            acc += partials[HPC * b + g]
        y[b] = acc + b_out.astype(np.float32)
    return y


_NC_CACHE = {}


def run(inputs, trace=False):
    """Returns (y_full [B,N,C] f32, exec_time_ns or None)."""
    x = np.asarray(inputs["x"], np.float32)
    w_qkv = np.asarray(inputs["w_qkv"], np.float32)
    b_qkv = np.asarray(inputs["b_qkv"], np.float32)
    w_out = np.asarray(inputs["w_out"], np.float32)
    b_out = np.asarray(inputs["b_out"], np.float32)

    if "nc" not in _NC_CACHE:
        _NC_CACHE["nc"] = build_bass()
    nc = _NC_CACHE["nc"]

    in_maps = [prep_core_inputs(x, w_qkv, b_qkv, w_out, core)
               for core in range(N_CORES)]
    res = run_bass_kernel_spmd(nc, in_maps, list(range(N_CORES)),
                               trace=trace)
    partials = [res.results[i]["y"] for i in range(N_CORES)]
    return assemble_output(partials, b_out), res.exec_time_ns


def kernel(**inputs):
    y, _ = run(inputs, trace=False)
    return y


# revision 27
# speedup vs baseline: 1.2513x; 1.2513x over previous
"""Trainium2 Bass kernel for CustomScaledDotProductAttention.

Full module: y = out_proj(softmax(q k^T / sqrt(D)) v) with fused qkv proj.
Shapes: x [2, 2048, 1024], H=16 heads, D=64.

Sharding (8 cores): core = b*4 + g, b = batch (2), g = head-group (4 heads).
Each core computes its batch's qkv projection restricted to its 4 heads,
attention for those heads, and the out-proj partial product (rows of
w_out.T owned by its heads).  Host sums the 4 partials per batch and adds
b_out (standard row-sharded tensor-parallel gather).

Device-side design notes:
  - All matmul operands are bf16 (1 cycle/row at any free-dim size, half
    the SBUF/DMA traffic and PE toggle power of f32).  Inputs are
    converted to bf16 on the host; PSUM accumulates f32.
  - qT/kT stored [d, n] (head-pairs packed in 128 partitions) so scores are
    computed TRANSPOSED (sT[m, n] = k q^T) -> softmax needs no transposes.
  - The two heads of a pair write one 2-bank PSUM tile ([128, 1024]); a
    SINGLE ScalarE exp instruction (fused *0.125) covers both banks,
    halving activation instruction overhead.  ScalarE is the phase-2
    pacing engine (~1.05us per m-step), so its overhead is the roofline.
  - V stored [m, d] with an all-ones 65th column per head; the attn@v
    matmul (M=65) then yields o^T[d, n] AND the softmax denominator l[n]
    (row 64) in a single pass of e^T through the PE.
  - attn@v emission LAGS the score/exp emission by 2 m-steps: the PE's
    in-order queue then never waits on ScalarE's exp, keeping the PE
    gapless so the HAM clock governor holds it at full p-state (the
    baseline spent all of phase 2 HAM-throttled to half clock).
  - Softmax normalization is off the PE critical path: o^T is copied out
    of PSUM by DVE (freeing the accumulator bank), then reciprocal /
    broadcast-DMA / multiply trail behind.
"""

import numpy as np
import ml_dtypes

import concourse.bass as bass
import concourse.mybir as mybir
import concourse.tile as tile
from concourse import library_config
from concourse.bass_utils import run_bass_kernel_spmd

F32 = mybir.dt.float32
BF16 = mybir.dt.bfloat16

B, N, C, H, D = 2, 2048, 1024, 16, 64
SCALE = D ** -0.5          # 0.125
HPC = 4                    # heads per core
N_CORES = 8
NK = C // 128              # 8 contraction chunks of 128
NM = N // 128              # 16 m-chunks
NN = N // 512              # 4 n-chunks of 512
VW = HPC * (D + 1)         # 260: v columns + ones column per head
LAG = 2                    # attn@v emission lag (m-steps) behind scores


def _emit(tc, nc, xT, wqk, bq, wv, wo, y, lbounce):
    PS = bass.MemorySpace.PSUM

    with (
        nc.allow_low_precision(reason="bf16 matmul operands; psum f32"),
        tc.tile_pool(name="persist", bufs=1) as pp,
        tc.tile_pool(name="qk", bufs=1) as qkp,
        tc.tile_pool(name="vp", bufs=1) as vp,
        tc.tile_pool(name="at", bufs=1) as atp,
    ):
        # ---- persistent tiles ----
        F32R = mybir.dt.float32r
        QK = [qkp.tile([128, N], BF16, tag=f"qk{j}", name=f"qk{j}")
              for j in range(4)]
        Vb = vp.tile([128, NM, VW], BF16, tag="vb")    # V[m] = Vb[:, m, :]
        # AT/wo stay f32r: DVE's bf16-out tensor_tensor is ~5x slower
        # (3.3us vs 0.7us per [64,512] mul), and f32r matmuls at N=512
        # stream at the same 1 cycle/row as bf16.
        AT = [atp.tile([128, N], F32R, tag=f"at{p}", name=f"at{p}")
              for p in range(2)]
        wo0 = pp.tile([128, 1024], F32R, tag="wo0")
        wo1 = pp.tile([128, 1024], F32R, tag="wo1")
        bqs = pp.tile([128, 4], F32, tag="bqs")
        nc.sync.dma_start(out=bqs, in_=bq[:, :])

        # ================= phase 1: projections =================
        with (
            tc.tile_pool(name="xw", bufs=1) as xw,
            tc.tile_pool(name="pps", bufs=8, space=PS) as pps,
        ):
            # chunked loads so matmuls can start before the full load lands
            xt = xw.tile([128, NK, N], BF16, tag="xt")
            wq = xw.tile([128, NK, 512], BF16, tag="wq")
            wvt = xw.tile([128, NK, VW], BF16, tag="wvt")
            for c in range(NK):
                nc.sync.dma_start(out=xt[:, c, :],
                                  in_=xT[c * 128:(c + 1) * 128, :])
                nc.sync.dma_start(out=wq[:, c, :],
                                  in_=wqk[c * 128:(c + 1) * 128, :])
                nc.sync.dma_start(out=wvt[:, c, :],
                                  in_=wv[c * 128:(c + 1) * 128, :])
            xt1f = xw.tile([1, N], F32, tag="xt1f")
            nc.vector.memset(xt1f, 1.0)
            xt1 = xw.tile([1, N], BF16, tag="xt1")
            nc.vector.tensor_copy(xt1, xt1f)
            wvb = xw.tile([1, VW], BF16, tag="wvb")
            nc.gpsimd.dma_start(out=wvb, in_=wv[C:C + 1, :])
            # wo loads early; consumed only in phase 3
            nc.sync.dma_start(out=wo0, in_=wo[0:128, :])
            nc.sync.dma_start(out=wo1, in_=wo[128:256, :])

            # qk-proj, contraction-chunk OUTER in two j-groups of 8 PSUM
            # chains each: the first matmuls only need DMA chunk 0, so
            # compute starts ~2us into the load instead of ~30us.
            # bias added by DVE during the PSUM->SBUF copy.
            for jg in range(2):
                ps = [[pps.tile([128, 512], F32, tag="pq", name="pq")
                       for n in range(NN)] for j in range(2)]
                for c in range(NK):
                    for j2 in range(2):
                        j = 2 * jg + j2
                        for n in range(NN):
                            nc.tensor.matmul(
                                ps[j2][n], wq[:, c, j * 128:(j + 1) * 128],
                                xt[:, c, n * 512:(n + 1) * 512],
                                start=(c == 0), stop=(c == NK - 1))
                for j2 in range(2):
                    j = 2 * jg + j2
                    for n in range(NN):
                        nc.vector.tensor_scalar_add(
                            QK[j][:, n * 512:(n + 1) * 512], ps[j2][n],
                            bqs[:, j:j + 1])

            # v-proj: V[m] = sum_c xt[c][:, m].T @ wvt[c] (+bias row)
            for m in range(NM):
                ps = pps.tile([128, VW], F32, tag="pq", name="pv")
                for c in range(NK):
                    nc.tensor.matmul(
                        ps, xt[:, c, m * 128:(m + 1) * 128], wvt[:, c, :],
                        start=(c == 0), stop=False)
                nc.tensor.matmul(
                    ps, xt1[:, m * 128:(m + 1) * 128], wvb,
                    start=False, stop=True)
                nc.scalar.copy(Vb[:, m, :], ps)

        # ========= phase 2: attention (+ out-proj on p=1) =========
        # otp's 4 banks are shared by the attn@v accumulators and the
        # out-proj accumulators: out-proj for n-chunk n is emitted right
        # after its normalization, so its matmuls fill the PE slack while
        # ScalarE works on the next n-chunk's exps, and y streams out of
        # PSUM over DMA with no SBUF staging.
        with (
            tc.tile_pool(name="sc", bufs=2, space=PS) as scp,
            tc.tile_pool(name="ot", bufs=4, space=PS) as otp,
            tc.tile_pool(name="et", bufs=3) as etp,
            tc.tile_pool(name="lv", bufs=4) as lvp,
            tc.tile_pool(name="orw", bufs=4) as orp,
            tc.tile_pool(name="bcs", bufs=4) as bcp,
        ):
            for p in range(2):            # head pairs
                Qt, Kt = QK[2 * p], QK[2 * p + 1]
                for n in range(NN):
                    ot = [otp.tile([128, 512], F32, tag="ot", name="ot")
                          for _ in range(2)]

                    def emit_av(m, e):
                        for hh in range(2):
                            hcol = (2 * p + hh) * (D + 1)
                            nc.tensor.matmul(
                                ot[hh][0:65, :],
                                Vb[:, m, hcol:hcol + 65],
                                e[:, hh * 512:(hh + 1) * 512],
                                start=(m == 0), stop=(m == NM - 1))

                    pend = []
                    for m in range(NM):
                        # both heads' scores into one 2-bank PSUM tile
                        sp = scp.tile([128, 1024], F32, tag="sc")
                        nc.tensor.matmul(
                            sp[:, 0:512], Kt[0:64, m * 128:(m + 1) * 128],
                            Qt[0:64, n * 512:(n + 1) * 512],
                            start=True, stop=True, tile_position=(0, 0))
                        nc.tensor.matmul(
                            sp[:, 512:1024],
                            Kt[64:128, m * 128:(m + 1) * 128],
                            Qt[64:128, n * 512:(n + 1) * 512],
                            start=True, stop=True, tile_position=(64, 0))
                        # single exp over both banks; bf16 out
                        e = etp.tile([128, 1024], BF16, tag="et")
                        nc.scalar.activation(
                            e, sp, mybir.ActivationFunctionType.Exp,
                            scale=SCALE)
                        pend.append((m, e))
                        if len(pend) > LAG:
                            emit_av(*pend.pop(0))
                    for me in pend:
                        emit_av(*me)

                    # evacuate PSUM promptly; normalization trails on
                    # DVE + bounce-DMAs, fully off the PE critical path
                    for hh in range(2):
                        orw = orp.tile([65, 512], F32, tag="orw",
                                       name="orw")
                        nc.vector.tensor_copy(orw, ot[hh][0:65, :])
                        linv = lvp.tile([1, 512], F32, tag="lv")
                        nc.vector.reciprocal(linv, orw[64:65, :])
                        idx = (p * NN + n) * 2 + hh
                        nc.gpsimd.dma_start(
                            out=lbounce[idx:idx + 1, :], in_=linv)
                        bc = bcp.tile([64, 512], F32, tag="bc", name="bc")
                        nc.gpsimd.dma_start(
                            out=bc,
                            in_=lbounce[idx:idx + 1, :]
                            .to_broadcast((64, 512)))
                        nc.vector.tensor_mul(
                            AT[p][hh * 64:(hh + 1) * 64,
                                  n * 512:(n + 1) * 512],
                            orw[0:64, :], bc)

        # ================= phase 3: out-proj =================
        # PSUM evacuation on DVE (ScalarE's copy rate would pace the
        # phase), y stores split across two DMA queues.
        with (
            tc.tile_pool(name="yps", bufs=4, space=PS) as ypsp,
            tc.tile_pool(name="ysb", bufs=4) as ysbp,
        ):
            for t in range(NM):
                yp = [ypsp.tile([128, 512], F32, tag="yp", name="yp")
                      for _ in range(2)]
                for ic, (a, w) in enumerate(((AT[0], wo0), (AT[1], wo1))):
                    for oc_ in range(2):
                        nc.tensor.matmul(
                            yp[oc_], a[:, t * 128:(t + 1) * 128],
                            w[:, oc_ * 512:(oc_ + 1) * 512],
                            start=(ic == 0), stop=(ic == 1))
                ysb = ysbp.tile([128, 1024], F32, tag="ysb")
                nc.vector.tensor_copy(ysb[:, 0:512], yp[0])
                nc.vector.tensor_copy(ysb[:, 512:1024], yp[1])
                eng = nc.sync if t % 2 == 0 else nc.gpsimd
                eng.dma_start(out=y[t * 128:(t + 1) * 128, :], in_=ysb)


def _split_multi_waits(nc):
    """Hoist all-but-one sem wait from instructions onto standalone
    EventSemaphore instructions: most TRN2 instruction encodings carry a
    single sync-wait slot (walrus: "Too many sync wait commands")."""
    import bass_rust
    nop_id = [0]
    for fn in nc.m.functions:
        for blk in fn.blocks:
            insts = blk.instructions
            out = []
            changed = False
            for ins in insts:
                si = ins.sync_info
                is_evsem = isinstance(ins, mybir.InstEventSemaphore)
                if (si is not None and si.on_wait is not None
                        and len(si.on_wait) > 1 and not is_evsem):
                    waits = list(si.on_wait)
                    for w in waits[:-1]:
                        ev = mybir.InstEventSemaphore(
                            name=f"waitev_{nop_id[0]}", engine=ins.engine)
                        nop_id[0] += 1
                        ev.sync_info = bass_rust.SyncInfo(
                            on_wait=[w], on_update=[])
                        out.append(ev)
                    ins.sync_info = bass_rust.SyncInfo(
                        on_wait=[waits[-1]],
                        on_update=list(si.on_update or []))
                    changed = True
                out.append(ins)
            if changed:
                blk.instructions = out


def build_bass(split_waits=True):
    nc = bass.Bass()
    xT = nc.dram_tensor("xT", [C, N], BF16, kind="ExternalInput")
    wqk = nc.dram_tensor("wqk", [C, 512], BF16, kind="ExternalInput")
    bq = nc.dram_tensor("bq", [128, 4], F32, kind="ExternalInput")
    wv = nc.dram_tensor("wv", [C + 1, VW], BF16, kind="ExternalInput")
    wo = nc.dram_tensor("wo", [2 * 128, 1024], mybir.dt.float32r,
                        kind="ExternalInput")
    y = nc.dram_tensor("y", [N, C], F32, kind="ExternalOutput")
    lbounce = nc.dram_tensor("lbounce", [16, 512], F32)
    with tile.TileContext(nc) as tc:
        _emit(tc, nc, xT, wqk, bq, wv, wo, y, lbounce)
    if split_waits:
        _split_multi_waits(nc)
    return nc


def prep_core_inputs(x, w_qkv, b_qkv, w_out, core):
    """Build the per-core input arrays (bf16 except the f32 qk bias)."""
    b, g = divmod(core, HPC)
    heads = [HPC * g + i for i in range(HPC)]
    bf = ml_dtypes.bfloat16

    xTa = np.ascontiguousarray(x[b].T.astype(bf))

    def q_rows(h):
        return w_qkv[h * D:(h + 1) * D]

    def k_rows(h):
        return w_qkv[C + h * D:C + (h + 1) * D]

    def v_rows(h):
        return w_qkv[2 * C + h * D:2 * C + (h + 1) * D]

    h0, h1, h2, h3 = heads
    wqk_rows = np.concatenate([
        q_rows(h0), q_rows(h1), k_rows(h0), k_rows(h1),
        q_rows(h2), q_rows(h3), k_rows(h2), k_rows(h3)], 0)   # [512, C]
    bqk = np.concatenate([
        b_qkv[h0 * D:(h0 + 1) * D], b_qkv[h1 * D:(h1 + 1) * D],
        b_qkv[C + h0 * D:C + (h0 + 1) * D],
        b_qkv[C + h1 * D:C + (h1 + 1) * D],
        b_qkv[h2 * D:(h2 + 1) * D], b_qkv[h3 * D:(h3 + 1) * D],
        b_qkv[C + h2 * D:C + (h2 + 1) * D],
        b_qkv[C + h3 * D:C + (h3 + 1) * D]], 0)               # [512]

    wv_aug = np.zeros((C + 1, VW), np.float32)
    for i, h in enumerate(heads):
        wv_aug[:C, i * (D + 1):i * (D + 1) + D] = v_rows(h).T
        wv_aug[C, i * (D + 1):i * (D + 1) + D] = \
            b_qkv[2 * C + h * D:2 * C + (h + 1) * D]
        wv_aug[C, i * (D + 1) + D] = 1.0

    woa = np.concatenate([w_out[:, h * D:(h + 1) * D].T for h in heads], 0)

    return {
        "xT": xTa,
        "wqk": np.ascontiguousarray(wqk_rows.T.astype(bf)),
        "bq": np.ascontiguousarray(
            bqk.reshape(4, 128).T.astype(np.float32)),
        "wv": np.ascontiguousarray(wv_aug.astype(bf)),
        "wo": np.ascontiguousarray(woa.astype(np.float32)),
    }


def assemble_output(partials, b_out):
    """partials: list of 8 [N, C] arrays (core order). Returns [B, N, C]."""
    y = np.empty((B, N, C), np.float32)
    for b in range(B):
        acc = partials[HPC * b].astype(np.float32).copy()
        for g in range(1, HPC):
            acc += partials[HPC * b + g]
        y[b] = acc + b_out.astype(np.float32)
    return y


_NC_CACHE = {}


def run(inputs, trace=False):
    """Returns (y_full [B,N,C] f32, exec_time_ns or None)."""
    x = np.asarray(inputs["x"], np.float32)
    w_qkv = np.asarray(inputs["w_qkv"], np.float32)
    b_qkv = np.asarray(inputs["b_qkv"], np.float32)
    w_out = np.asarray(inputs["w_out"], np.float32)
    b_out = np.asarray(inputs["b_out"], np.float32)

    if "nc" not in _NC_CACHE:
        _NC_CACHE["nc"] = build_bass()
    nc = _NC_CACHE["nc"]

    in_maps = [prep_core_inputs(x, w_qkv, b_qkv, w_out, core)
               for core in range(N_CORES)]
    res = run_bass_kernel_spmd(nc, in_maps, list(range(N_CORES)),
                               trace=trace)
    partials = [res.results[i]["y"] for i in range(N_CORES)]
    return assemble_output(partials, b_out), res.exec_time_ns


def kernel(**inputs):
    y, _ = run(inputs, trace=False)
    return y


# revision 29
# speedup vs baseline: 1.3050x; 1.0430x over previous
"""Trainium2 Bass kernel for CustomScaledDotProductAttention.

Full module: y = out_proj(softmax(q k^T / sqrt(D)) v) with fused qkv proj.
Shapes: x [2, 2048, 1024], H=16 heads, D=64.

Sharding (8 cores): core = b*4 + g, b = batch (2), g = head-group (4 heads).
Each core computes its batch's qkv projection restricted to its 4 heads,
attention for those heads, and the out-proj partial product (rows of
w_out.T owned by its heads).  Host sums the 4 partials per batch and adds
b_out (standard row-sharded tensor-parallel gather).

Device-side design notes:
  - All matmul operands are bf16 (1 cycle/row at any free-dim size, half
    the SBUF/DMA traffic and PE toggle power of f32).  Inputs are
    converted to bf16 on the host; PSUM accumulates f32.
  - qT/kT stored [d, n] (head-pairs packed in 128 partitions) so scores are
    computed TRANSPOSED (sT[m, n] = k q^T) -> softmax needs no transposes.
  - The two heads of a pair write one 2-bank PSUM tile ([128, 1024]); a
    SINGLE ScalarE exp instruction (fused *0.125) covers both banks,
    halving activation instruction overhead.  ScalarE is the phase-2
    pacing engine (~1.05us per m-step), so its overhead is the roofline.
  - V stored [m, d] with an all-ones 65th column per head; the attn@v
    matmul (M=65) then yields o^T[d, n] AND the softmax denominator l[n]
    (row 64) in a single pass of e^T through the PE.
  - attn@v emission LAGS the score/exp emission by 2 m-steps: the PE's
    in-order queue then never waits on ScalarE's exp, keeping the PE
    gapless so the HAM clock governor holds it at full p-state (the
    baseline spent all of phase 2 HAM-throttled to half clock).
  - Softmax normalization is off the PE critical path: o^T is copied out
    of PSUM by DVE (freeing the accumulator bank), then reciprocal /
    broadcast-DMA / multiply trail behind.
"""

import numpy as np
import ml_dtypes

import concourse.bass as bass
import concourse.mybir as mybir
import concourse.tile as tile
from concourse import library_config
from concourse.bass_utils import run_bass_kernel_spmd

F32 = mybir.dt.float32
BF16 = mybir.dt.bfloat16

B, N, C, H, D = 2, 2048, 1024, 16, 64
SCALE = D ** -0.5          # 0.125
HPC = 4                    # heads per core
N_CORES = 8
NK = C // 128              # 8 contraction chunks of 128
NM = N // 128              # 16 m-chunks
NN = N // 512              # 4 n-chunks of 512
VW = HPC * (D + 1)         # 260: v columns + ones column per head
LAG = 2                    # attn@v emission lag (m-steps) behind scores


def _emit(tc, nc, xT, wqk, bq, wv, wo, y, lbounce):
    PS = bass.MemorySpace.PSUM

    with (
        nc.allow_low_precision(reason="bf16 matmul operands; psum f32"),
        tc.tile_pool(name="persist", bufs=1) as pp,
        tc.tile_pool(name="qk", bufs=1) as qkp,
        tc.tile_pool(name="vp", bufs=1) as vp,
        tc.tile_pool(name="at", bufs=1) as atp,
    ):
        # ---- persistent tiles ----
        F32R = mybir.dt.float32r
        QK = [qkp.tile([128, N], BF16, tag=f"qk{j}", name=f"qk{j}")
              for j in range(4)]
        Vb = vp.tile([128, NM, VW], BF16, tag="vb")    # V[m] = Vb[:, m, :]
        # AT/wo stay f32r: DVE's bf16-out tensor_tensor is ~5x slower
        # (3.3us vs 0.7us per [64,512] mul), and f32r matmuls at N=512
        # stream at the same 1 cycle/row as bf16.
        AT = [atp.tile([128, N], F32R, tag=f"at{p}", name=f"at{p}")
              for p in range(2)]
        wo0 = pp.tile([128, 1024], F32R, tag="wo0")
        wo1 = pp.tile([128, 1024], F32R, tag="wo1")
        bqs = pp.tile([128, 4], F32, tag="bqs")
        nc.sync.dma_start(out=bqs, in_=bq[:, :])

        # ================= phase 1: projections =================
        with (
            tc.tile_pool(name="xw", bufs=1) as xw,
            tc.tile_pool(name="pps", bufs=8, space=PS) as pps,
        ):
            # chunked loads so matmuls can start before the full load lands
            xt = xw.tile([128, NK, N], BF16, tag="xt")
            wq = xw.tile([128, NK, 512], BF16, tag="wq")
            wvt = xw.tile([128, NK, VW], BF16, tag="wvt")
            for c in range(NK):
                nc.sync.dma_start(out=xt[:, c, :],
                                  in_=xT[c * 128:(c + 1) * 128, :])
                nc.sync.dma_start(out=wq[:, c, :],
                                  in_=wqk[c * 128:(c + 1) * 128, :])
                nc.sync.dma_start(out=wvt[:, c, :],
                                  in_=wv[c * 128:(c + 1) * 128, :])
            xt1f = xw.tile([1, N], F32, tag="xt1f")
            nc.vector.memset(xt1f, 1.0)
            xt1 = xw.tile([1, N], BF16, tag="xt1")
            nc.vector.tensor_copy(xt1, xt1f)
            wvb = xw.tile([1, VW], BF16, tag="wvb")
            nc.gpsimd.dma_start(out=wvb, in_=wv[C:C + 1, :])
            # wo loads early; consumed only in phase 3
            nc.sync.dma_start(out=wo0, in_=wo[0:128, :])
            nc.sync.dma_start(out=wo1, in_=wo[128:256, :])

            # qk-proj, contraction-chunk OUTER in two j-groups of 8 PSUM
            # chains each: the first matmuls only need DMA chunk 0, so
            # compute starts ~2us into the load instead of ~30us.
            # bias added by DVE during the PSUM->SBUF copy.
            for jg in range(2):
                ps = [[pps.tile([128, 512], F32, tag="pq", name="pq")
                       for n in range(NN)] for j in range(2)]
                for c in range(NK):
                    for j2 in range(2):
                        j = 2 * jg + j2
                        for n in range(NN):
                            nc.tensor.matmul(
                                ps[j2][n], wq[:, c, j * 128:(j + 1) * 128],
                                xt[:, c, n * 512:(n + 1) * 512],
                                start=(c == 0), stop=(c == NK - 1))
                for j2 in range(2):
                    j = 2 * jg + j2
                    for n in range(NN):
                        nc.vector.tensor_scalar_add(
                            QK[j][:, n * 512:(n + 1) * 512], ps[j2][n],
                            bqs[:, j:j + 1])

            # v-proj: V[m] = sum_c xt[c][:, m].T @ wvt[c] (+bias row)
            for m in range(NM):
                ps = pps.tile([128, VW], F32, tag="pq", name="pv")
                for c in range(NK):
                    nc.tensor.matmul(
                        ps, xt[:, c, m * 128:(m + 1) * 128], wvt[:, c, :],
                        start=(c == 0), stop=False)
                nc.tensor.matmul(
                    ps, xt1[:, m * 128:(m + 1) * 128], wvb,
                    start=False, stop=True)
                nc.scalar.copy(Vb[:, m, :], ps)

        # ========= phase 2: attention (+ out-proj on p=1) =========
        # otp's 4 banks are shared by the attn@v accumulators and the
        # out-proj accumulators: out-proj for n-chunk n is emitted right
        # after its normalization, so its matmuls fill the PE slack while
        # ScalarE works on the next n-chunk's exps, and y streams out of
        # PSUM over DMA with no SBUF staging.
        with (
            tc.tile_pool(name="sc", bufs=2, space=PS) as scp,
            tc.tile_pool(name="ot", bufs=4, space=PS) as otp,
            tc.tile_pool(name="et", bufs=3) as etp,
            tc.tile_pool(name="lv", bufs=4) as lvp,
            tc.tile_pool(name="orw", bufs=4) as orp,
            tc.tile_pool(name="bcs", bufs=4) as bcp,
        ):
            for p in range(2):            # head pairs
                Qt, Kt = QK[2 * p], QK[2 * p + 1]
                for n in range(NN):
                    ot = [otp.tile([128, 512], F32, tag="ot", name="ot")
                          for _ in range(2)]

                    def emit_av(m, e):
                        for hh in range(2):
                            hcol = (2 * p + hh) * (D + 1)
                            nc.tensor.matmul(
                                ot[hh][0:65, :],
                                Vb[:, m, hcol:hcol + 65],
                                e[:, hh * 512:(hh + 1) * 512],
                                start=(m == 0), stop=(m == NM - 1))

                    pend = []
                    for m in range(NM):
                        # both heads' scores into one 2-bank PSUM tile
                        sp = scp.tile([128, 1024], F32, tag="sc")
                        nc.tensor.matmul(
                            sp[:, 0:512], Kt[0:64, m * 128:(m + 1) * 128],
                            Qt[0:64, n * 512:(n + 1) * 512],
                            start=True, stop=True, tile_position=(0, 0))
                        nc.tensor.matmul(
                            sp[:, 512:1024],
                            Kt[64:128, m * 128:(m + 1) * 128],
                            Qt[64:128, n * 512:(n + 1) * 512],
                            start=True, stop=True, tile_position=(64, 0))
                        # single exp over both banks; bf16 out
                        e = etp.tile([128, 1024], BF16, tag="et")
                        nc.scalar.activation(
                            e, sp, mybir.ActivationFunctionType.Exp,
                            scale=SCALE)
                        pend.append((m, e))
                        if len(pend) > LAG:
                            emit_av(*pend.pop(0))
                    for me in pend:
                        emit_av(*me)

                    # evacuate PSUM promptly; normalization trails on
                    # DVE + bounce-DMAs, fully off the PE critical path
                    for hh in range(2):
                        orw = orp.tile([65, 512], F32, tag="orw",
                                       name="orw")
                        nc.vector.tensor_copy(orw, ot[hh][0:65, :])
                        linv = lvp.tile([1, 512], F32, tag="lv")
                        nc.vector.reciprocal(linv, orw[64:65, :])
                        idx = (p * NN + n) * 2 + hh
                        nc.gpsimd.dma_start(
                            out=lbounce[idx:idx + 1, :], in_=linv)
                        bc = bcp.tile([64, 512], F32, tag="bc", name="bc")
                        nc.gpsimd.dma_start(
                            out=bc,
                            in_=lbounce[idx:idx + 1, :]
                            .to_broadcast((64, 512)))
                        nc.vector.tensor_mul(
                            AT[p][hh * 64:(hh + 1) * 64,
                                  n * 512:(n + 1) * 512],
                            orw[0:64, :], bc)

        # ================= phase 3: out-proj =================
        # y partials ship as bf16 (host sums in f32), halving store
        # traffic.  PSUM evacuation alternates ScalarE/DVE: the DVE
        # queue is still draining the last n-chunk's bounce-gated
        # normalization muls, so scalar-side copies keep the yp ring
        # moving.  Stores split across two DMA queues.
        with (
            tc.tile_pool(name="yps", bufs=4, space=PS) as ypsp,
            tc.tile_pool(name="ysb", bufs=4) as ysbp,
        ):
            for t in range(NM):
                yp = [ypsp.tile([128, 512], F32, tag="yp", name="yp")
                      for _ in range(2)]
                for ic, (a, w) in enumerate(((AT[0], wo0), (AT[1], wo1))):
                    for oc_ in range(2):
                        nc.tensor.matmul(
                            yp[oc_], a[:, t * 128:(t + 1) * 128],
                            w[:, oc_ * 512:(oc_ + 1) * 512],
                            start=(ic == 0), stop=(ic == 1))
                ysb = ysbp.tile([128, 1024], BF16, tag="ysb")
                if t % 2 == 0:
                    nc.scalar.copy(ysb[:, 0:512], yp[0])
                    nc.scalar.copy(ysb[:, 512:1024], yp[1])
                else:
                    nc.vector.tensor_copy(ysb[:, 0:512], yp[0])
                    nc.vector.tensor_copy(ysb[:, 512:1024], yp[1])
                eng = nc.sync if t % 2 == 0 else nc.gpsimd
                eng.dma_start(out=y[t * 128:(t + 1) * 128, :], in_=ysb)


def _split_multi_waits(nc):
    """Hoist all-but-one sem wait from instructions onto standalone
    EventSemaphore instructions: most TRN2 instruction encodings carry a
    single sync-wait slot (walrus: "Too many sync wait commands")."""
    import bass_rust
    nop_id = [0]
    for fn in nc.m.functions:
        for blk in fn.blocks:
            insts = blk.instructions
            out = []
            changed = False
            for ins in insts:
                si = ins.sync_info
                is_evsem = isinstance(ins, mybir.InstEventSemaphore)
                if (si is not None and si.on_wait is not None
                        and len(si.on_wait) > 1 and not is_evsem):
                    waits = list(si.on_wait)
                    for w in waits[:-1]:
                        ev = mybir.InstEventSemaphore(
                            name=f"waitev_{nop_id[0]}", engine=ins.engine)
                        nop_id[0] += 1
                        ev.sync_info = bass_rust.SyncInfo(
                            on_wait=[w], on_update=[])
                        out.append(ev)
                    ins.sync_info = bass_rust.SyncInfo(
                        on_wait=[waits[-1]],
                        on_update=list(si.on_update or []))
                    changed = True
                out.append(ins)
            if changed:
                blk.instructions = out


def build_bass(split_waits=True):
    nc = bass.Bass()
    xT = nc.dram_tensor("xT", [C, N], BF16, kind="ExternalInput")
    wqk = nc.dram_tensor("wqk", [C, 512], BF16, kind="ExternalInput")
    bq = nc.dram_tensor("bq", [128, 4], F32, kind="ExternalInput")
    wv = nc.dram_tensor("wv", [C + 1, VW], BF16, kind="ExternalInput")
    wo = nc.dram_tensor("wo", [2 * 128, 1024], mybir.dt.float32r,
                        kind="ExternalInput")
    y = nc.dram_tensor("y", [N, C], BF16, kind="ExternalOutput")
    lbounce = nc.dram_tensor("lbounce", [16, 512], F32)
    with tile.TileContext(nc) as tc:
        _emit(tc, nc, xT, wqk, bq, wv, wo, y, lbounce)
    if split_waits:
        _split_multi_waits(nc)
    return nc


def prep_core_inputs(x, w_qkv, b_qkv, w_out, core):
    """Build the per-core input arrays (bf16 except the f32 qk bias)."""
    b, g = divmod(core, HPC)
    heads = [HPC * g + i for i in range(HPC)]
    bf = ml_dtypes.bfloat16

    xTa = np.ascontiguousarray(x[b].T.astype(bf))

    def q_rows(h):
        return w_qkv[h * D:(h + 1) * D]

    def k_rows(h):
        return w_qkv[C + h * D:C + (h + 1) * D]

    def v_rows(h):
        return w_qkv[2 * C + h * D:2 * C + (h + 1) * D]

    h0, h1, h2, h3 = heads
    wqk_rows = np.concatenate([
        q_rows(h0), q_rows(h1), k_rows(h0), k_rows(h1),
        q_rows(h2), q_rows(h3), k_rows(h2), k_rows(h3)], 0)   # [512, C]
    bqk = np.concatenate([
        b_qkv[h0 * D:(h0 + 1) * D], b_qkv[h1 * D:(h1 + 1) * D],
        b_qkv[C + h0 * D:C + (h0 + 1) * D],
        b_qkv[C + h1 * D:C + (h1 + 1) * D],
        b_qkv[h2 * D:(h2 + 1) * D], b_qkv[h3 * D:(h3 + 1) * D],
        b_qkv[C + h2 * D:C + (h2 + 1) * D],
        b_qkv[C + h3 * D:C + (h3 + 1) * D]], 0)               # [512]

    wv_aug = np.zeros((C + 1, VW), np.float32)
    for i, h in enumerate(heads):
        wv_aug[:C, i * (D + 1):i * (D + 1) + D] = v_rows(h).T
        wv_aug[C, i * (D + 1):i * (D + 1) + D] = \
            b_qkv[2 * C + h * D:2 * C + (h + 1) * D]
        wv_aug[C, i * (D + 1) + D] = 1.0

    woa = np.concatenate([w_out[:, h * D:(h + 1) * D].T for h in heads], 0)

    return {
        "xT": xTa,
        "wqk": np.ascontiguousarray(wqk_rows.T.astype(bf)),
        "bq": np.ascontiguousarray(
            bqk.reshape(4, 128).T.astype(np.float32)),
        "wv": np.ascontiguousarray(wv_aug.astype(bf)),
        "wo": np.ascontiguousarray(woa.astype(np.float32)),
    }


def assemble_output(partials, b_out):
    """partials: list of 8 [N, C] arrays (core order). Returns [B, N, C]."""
    y = np.empty((B, N, C), np.float32)
    for b in range(B):
        acc = partials[HPC * b].astype(np.float32).copy()
        for g in range(1, HPC):
            acc += partials[HPC * b + g]
        y[b] = acc + b_out.astype(np.float32)
    return y


_NC_CACHE = {}


def run(inputs, trace=False):
    """Returns (y_full [B,N,C] f32, exec_time_ns or None)."""
    x = np.asarray(inputs["x"], np.float32)
    w_qkv = np.asarray(inputs["w_qkv"], np.float32)
    b_qkv = np.asarray(inputs["b_qkv"], np.float32)
    w_out = np.asarray(inputs["w_out"], np.float32)
    b_out = np.asarray(inputs["b_out"], np.float32)

    if "nc" not in _NC_CACHE:
        _NC_CACHE["nc"] = build_bass()
    nc = _NC_CACHE["nc"]

    in_maps = [prep_core_inputs(x, w_qkv, b_qkv, w_out, core)
               for core in range(N_CORES)]
    res = run_bass_kernel_spmd(nc, in_maps, list(range(N_CORES)),
                               trace=trace)
    partials = [res.results[i]["y"] for i in range(N_CORES)]
    return assemble_output(partials, b_out), res.exec_time_ns


def kernel(**inputs):
    y, _ = run(inputs, trace=False)
    return y


# revision 32
# speedup vs baseline: 1.3085x; 1.0026x over previous
"""Trainium2 Bass kernel for CustomScaledDotProductAttention.

Full module: y = out_proj(softmax(q k^T / sqrt(D)) v) with fused qkv proj.
Shapes: x [2, 2048, 1024], H=16 heads, D=64.

Sharding (8 cores): core = b*4 + g, b = batch (2), g = head-group (4 heads).
Each core computes its batch's qkv projection restricted to its 4 heads,
attention for those heads, and the out-proj partial product (rows of
w_out.T owned by its heads).  Host sums the 4 partials per batch and adds
b_out (standard row-sharded tensor-parallel gather).

Device-side design notes:
  - All matmul operands are bf16 (1 cycle/row at any free-dim size, half
    the SBUF/DMA traffic and PE toggle power of f32).  Inputs are
    converted to bf16 on the host; PSUM accumulates f32.
  - qT/kT stored [d, n] (head-pairs packed in 128 partitions) so scores are
    computed TRANSPOSED (sT[m, n] = k q^T) -> softmax needs no transposes.
  - The two heads of a pair write one 2-bank PSUM tile ([128, 1024]); a
    SINGLE ScalarE exp instruction (fused *0.125) covers both banks,
    halving activation instruction overhead.  ScalarE is the phase-2
    pacing engine (~1.05us per m-step), so its overhead is the roofline.
  - V stored [m, d] with an all-ones 65th column per head; the attn@v
    matmul (M=65) then yields o^T[d, n] AND the softmax denominator l[n]
    (row 64) in a single pass of e^T through the PE.
  - attn@v emission LAGS the score/exp emission by 2 m-steps: the PE's
    in-order queue then never waits on ScalarE's exp, keeping the PE
    gapless so the HAM clock governor holds it at full p-state (the
    baseline spent all of phase 2 HAM-throttled to half clock).
  - Softmax normalization is off the PE critical path: o^T is copied out
    of PSUM by DVE (freeing the accumulator bank), then reciprocal /
    broadcast-DMA / multiply trail behind.
"""

import numpy as np
import ml_dtypes

import concourse.bass as bass
import concourse.mybir as mybir
import concourse.tile as tile
from concourse import library_config
from concourse.bass_utils import run_bass_kernel_spmd

F32 = mybir.dt.float32
BF16 = mybir.dt.bfloat16

B, N, C, H, D = 2, 2048, 1024, 16, 64
SCALE = D ** -0.5          # 0.125
HPC = 4                    # heads per core
N_CORES = 8
NK = C // 128              # 8 contraction chunks of 128
NM = N // 128              # 16 m-chunks
NN = N // 512              # 4 n-chunks of 512
VW = HPC * (D + 1)         # 260: v columns + ones column per head
LAG = 2                    # attn@v emission lag (m-steps) behind scores


def _emit(tc, nc, xT, wqk, bq, wv, wo, y, lbounce):
    PS = bass.MemorySpace.PSUM

    with (
        nc.allow_low_precision(reason="bf16 matmul operands; psum f32"),
        tc.tile_pool(name="persist", bufs=1) as pp,
        tc.tile_pool(name="qk", bufs=1) as qkp,
        tc.tile_pool(name="vp", bufs=1) as vp,
        tc.tile_pool(name="at", bufs=1) as atp,
    ):
        # ---- persistent tiles ----
        F32R = mybir.dt.float32r
        QK = [qkp.tile([128, N], BF16, tag=f"qk{j}", name=f"qk{j}")
              for j in range(4)]
        Vb = vp.tile([128, NM, VW], BF16, tag="vb")    # V[m] = Vb[:, m, :]
        # AT/wo stay f32r: DVE's bf16-out tensor_tensor is ~5x slower
        # (3.3us vs 0.7us per [64,512] mul), and f32r matmuls at N=512
        # stream at the same 1 cycle/row as bf16.
        AT = [atp.tile([128, N], F32R, tag=f"at{p}", name=f"at{p}")
              for p in range(2)]
        wo0 = pp.tile([128, 1024], F32R, tag="wo0")
        wo1 = pp.tile([128, 1024], F32R, tag="wo1")
        bqs = pp.tile([128, 4], F32, tag="bqs")
        nc.sync.dma_start(out=bqs, in_=bq[:, :])

        # ================= phase 1: projections =================
        with (
            tc.tile_pool(name="xw", bufs=1) as xw,
            tc.tile_pool(name="pps", bufs=8, space=PS) as pps,
        ):
            # chunked loads so matmuls can start before the full load lands
            xt = xw.tile([128, NK, N], BF16, tag="xt")
            wq = xw.tile([128, NK, 512], BF16, tag="wq")
            wvt = xw.tile([128, NK, VW], BF16, tag="wvt")
            for c in range(NK):
                nc.sync.dma_start(out=xt[:, c, :],
                                  in_=xT[c * 128:(c + 1) * 128, :])
                nc.sync.dma_start(out=wq[:, c, :],
                                  in_=wqk[c * 128:(c + 1) * 128, :])
                nc.sync.dma_start(out=wvt[:, c, :],
                                  in_=wv[c * 128:(c + 1) * 128, :])
            xt1f = xw.tile([1, N], F32, tag="xt1f")
            nc.vector.memset(xt1f, 1.0)
            xt1 = xw.tile([1, N], BF16, tag="xt1")
            nc.vector.tensor_copy(xt1, xt1f)
            wvb = xw.tile([1, VW], BF16, tag="wvb")
            nc.gpsimd.dma_start(out=wvb, in_=wv[C:C + 1, :])
            # wo loads early; consumed only in phase 3
            nc.sync.dma_start(out=wo0, in_=wo[0:128, :])
            nc.sync.dma_start(out=wo1, in_=wo[128:256, :])

            # qk-proj, contraction-chunk OUTER in two j-groups of 8 PSUM
            # chains each: the first matmuls only need DMA chunk 0, so
            # compute starts ~2us into the load instead of ~30us.
            # bias added by DVE during the PSUM->SBUF copy.
            for jg in range(2):
                ps = [[pps.tile([128, 512], F32, tag="pq", name="pq")
                       for n in range(NN)] for j in range(2)]
                for c in range(NK):
                    for j2 in range(2):
                        j = 2 * jg + j2
                        for n in range(NN):
                            nc.tensor.matmul(
                                ps[j2][n], wq[:, c, j * 128:(j + 1) * 128],
                                xt[:, c, n * 512:(n + 1) * 512],
                                start=(c == 0), stop=(c == NK - 1))
                for j2 in range(2):
                    j = 2 * jg + j2
                    for n in range(NN):
                        nc.vector.tensor_scalar_add(
                            QK[j][:, n * 512:(n + 1) * 512], ps[j2][n],
                            bqs[:, j:j + 1])

            # v-proj: V[m] = sum_c xt[c][:, m].T @ wvt[c] (+bias row)
            for m in range(NM):
                ps = pps.tile([128, VW], F32, tag="pq", name="pv")
                for c in range(NK):
                    nc.tensor.matmul(
                        ps, xt[:, c, m * 128:(m + 1) * 128], wvt[:, c, :],
                        start=(c == 0), stop=False)
                nc.tensor.matmul(
                    ps, xt1[:, m * 128:(m + 1) * 128], wvb,
                    start=False, stop=True)
                nc.scalar.copy(Vb[:, m, :], ps)

        # ========= phase 2: attention (+ out-proj on p=1) =========
        # otp's 4 banks are shared by the attn@v accumulators and the
        # out-proj accumulators: out-proj for n-chunk n is emitted right
        # after its normalization, so its matmuls fill the PE slack while
        # ScalarE works on the next n-chunk's exps, and y streams out of
        # PSUM over DMA with no SBUF staging.
        with (
            tc.tile_pool(name="sc", bufs=2, space=PS) as scp,
            tc.tile_pool(name="ot", bufs=4, space=PS) as otp,
            tc.tile_pool(name="et", bufs=3) as etp,
            tc.tile_pool(name="lv", bufs=4) as lvp,
            tc.tile_pool(name="orw", bufs=4) as orp,
            tc.tile_pool(name="bcs", bufs=4) as bcp,
        ):
            mul_tail = []   # last n-chunk's muls, deferred into phase 3

            for p in range(2):            # head pairs
                Qt, Kt = QK[2 * p], QK[2 * p + 1]
                for n in range(NN):
                    ot = [otp.tile([128, 512], F32, tag="ot", name="ot")
                          for _ in range(2)]

                    def emit_av(m, e):
                        for hh in range(2):
                            hcol = (2 * p + hh) * (D + 1)
                            nc.tensor.matmul(
                                ot[hh][0:65, :],
                                Vb[:, m, hcol:hcol + 65],
                                e[:, hh * 512:(hh + 1) * 512],
                                start=(m == 0), stop=(m == NM - 1))

                    pend = []
                    for m in range(NM):
                        # both heads' scores into one 2-bank PSUM tile
                        sp = scp.tile([128, 1024], F32, tag="sc")
                        nc.tensor.matmul(
                            sp[:, 0:512], Kt[0:64, m * 128:(m + 1) * 128],
                            Qt[0:64, n * 512:(n + 1) * 512],
                            start=True, stop=True, tile_position=(0, 0))
                        nc.tensor.matmul(
                            sp[:, 512:1024],
                            Kt[64:128, m * 128:(m + 1) * 128],
                            Qt[64:128, n * 512:(n + 1) * 512],
                            start=True, stop=True, tile_position=(64, 0))
                        # single exp over both banks; bf16 out
                        e = etp.tile([128, 1024], BF16, tag="et")
                        nc.scalar.activation(
                            e, sp, mybir.ActivationFunctionType.Exp,
                            scale=SCALE)
                        pend.append((m, e))
                        if len(pend) > LAG:
                            emit_av(*pend.pop(0))
                    for me in pend:
                        emit_av(*me)

                    # evacuate PSUM promptly; normalization trails on
                    # DVE + bounce-DMAs, fully off the PE critical path
                    for hh in range(2):
                        orw = orp.tile([65, 512], F32, tag="orw",
                                       name="orw")
                        nc.vector.tensor_copy(orw, ot[hh][0:65, :])
                        linv = lvp.tile([1, 512], F32, tag="lv")
                        nc.vector.reciprocal(linv, orw[64:65, :])
                        idx = (p * NN + n) * 2 + hh
                        nc.gpsimd.dma_start(
                            out=lbounce[idx:idx + 1, :], in_=linv)
                        bc = bcp.tile([64, 512], F32, tag="bc", name="bc")
                        nc.gpsimd.dma_start(
                            out=bc,
                            in_=lbounce[idx:idx + 1, :]
                            .to_broadcast((64, 512)))
                        if p == 1 and n == NN - 1:
                            # defer: these muls wait ~5us on the bounce
                            # broadcast; emitted now they'd block the DVE
                            # queue ahead of phase 3's ysb copies (yp
                            # ring stalls -> PE idle -> HAM throttle).
                            # Their only consumers are the t=12..15
                            # out-proj chains.
                            mul_tail.append((hh, orw, bc))
                        else:
                            nc.vector.tensor_mul(
                                AT[p][hh * 64:(hh + 1) * 64,
                                      n * 512:(n + 1) * 512],
                                orw[0:64, :], bc)

        # ================= phase 3: out-proj =================
        # y partials ship as bf16 (host sums in f32), halving store
        # traffic.  PSUM evacuation alternates ScalarE/DVE: the DVE
        # queue is still draining the last n-chunk's bounce-gated
        # normalization muls, so scalar-side copies keep the yp ring
        # moving.  Stores split across two DMA queues.
        with (
            tc.tile_pool(name="yps", bufs=4, space=PS) as ypsp,
            tc.tile_pool(name="ysb", bufs=4) as ysbp,
        ):
            for t in range(NM):
                if t == 8:
                    # bounce broadcasts have had ~10us to land; the DVE
                    # queue here holds only ysb copies, so these never
                    # block anything.  t=12..15 chains consume them.
                    for hh, orw, bc in mul_tail:
                        nc.vector.tensor_mul(
                            AT[1][hh * 64:(hh + 1) * 64,
                                  (NN - 1) * 512:NN * 512],
                            orw[0:64, :], bc)
                    mul_tail.clear()
                yp = [ypsp.tile([128, 512], F32, tag="yp", name="yp")
                      for _ in range(2)]
                for ic, (a, w) in enumerate(((AT[0], wo0), (AT[1], wo1))):
                    for oc_ in range(2):
                        nc.tensor.matmul(
                            yp[oc_], a[:, t * 128:(t + 1) * 128],
                            w[:, oc_ * 512:(oc_ + 1) * 512],
                            start=(ic == 0), stop=(ic == 1))
                ysb = ysbp.tile([128, 1024], BF16, tag="ysb")
                if t % 2 == 0:
                    nc.scalar.copy(ysb[:, 0:512], yp[0])
                    nc.scalar.copy(ysb[:, 512:1024], yp[1])
                else:
                    nc.vector.tensor_copy(ysb[:, 0:512], yp[0])
                    nc.vector.tensor_copy(ysb[:, 512:1024], yp[1])
                eng = nc.sync if t % 2 == 0 else nc.gpsimd
                eng.dma_start(out=y[t * 128:(t + 1) * 128, :], in_=ysb)


def _split_multi_waits(nc):
    """Hoist all-but-one sem wait from instructions onto standalone
    EventSemaphore instructions: most TRN2 instruction encodings carry a
    single sync-wait slot (walrus: "Too many sync wait commands")."""
    import bass_rust
    nop_id = [0]
    for fn in nc.m.functions:
        for blk in fn.blocks:
            insts = blk.instructions
            out = []
            changed = False
            for ins in insts:
                si = ins.sync_info
                is_evsem = isinstance(ins, mybir.InstEventSemaphore)
                if (si is not None and si.on_wait is not None
                        and len(si.on_wait) > 1 and not is_evsem):
                    waits = list(si.on_wait)
                    for w in waits[:-1]:
                        ev = mybir.InstEventSemaphore(
                            name=f"waitev_{nop_id[0]}", engine=ins.engine)
                        nop_id[0] += 1
                        ev.sync_info = bass_rust.SyncInfo(
                            on_wait=[w], on_update=[])
                        out.append(ev)
                    ins.sync_info = bass_rust.SyncInfo(
                        on_wait=[waits[-1]],
                        on_update=list(si.on_update or []))
                    changed = True
                out.append(ins)
            if changed:
                blk.instructions = out


def build_bass(split_waits=True):
    nc = bass.Bass()
    xT = nc.dram_tensor("xT", [C, N], BF16, kind="ExternalInput")
    wqk = nc.dram_tensor("wqk", [C, 512], BF16, kind="ExternalInput")
    bq = nc.dram_tensor("bq", [128, 4], F32, kind="ExternalInput")
    wv = nc.dram_tensor("wv", [C + 1, VW], BF16, kind="ExternalInput")
    wo = nc.dram_tensor("wo", [2 * 128, 1024], mybir.dt.float32r,
                        kind="ExternalInput")
    y = nc.dram_tensor("y", [N, C], BF16, kind="ExternalOutput")
    lbounce = nc.dram_tensor("lbounce", [16, 512], F32)
    with tile.TileContext(nc) as tc:
        _emit(tc, nc, xT, wqk, bq, wv, wo, y, lbounce)
    if split_waits:
        _split_multi_waits(nc)
    return nc


def prep_core_inputs(x, w_qkv, b_qkv, w_out, core):
    """Build the per-core input arrays (bf16 except the f32 qk bias)."""
    b, g = divmod(core, HPC)
    heads = [HPC * g + i for i in range(HPC)]
    bf = ml_dtypes.bfloat16

    xTa = np.ascontiguousarray(x[b].T.astype(bf))

    def q_rows(h):
        return w_qkv[h * D:(h + 1) * D]

    def k_rows(h):
        return w_qkv[C + h * D:C + (h + 1) * D]

    def v_rows(h):
        return w_qkv[2 * C + h * D:2 * C + (h + 1) * D]

    h0, h1, h2, h3 = heads
    wqk_rows = np.concatenate([
        q_rows(h0), q_rows(h1), k_rows(h0), k_rows(h1),
        q_rows(h2), q_rows(h3), k_rows(h2), k_rows(h3)], 0)   # [512, C]
    bqk = np.concatenate([
        b_qkv[h0 * D:(h0 + 1) * D], b_qkv[h1 * D:(h1 + 1) * D],
        b_qkv[C + h0 * D:C + (h0 + 1) * D],
        b_qkv[C + h1 * D:C + (h1 + 1) * D],
        b_qkv[h2 * D:(h2 + 1) * D], b_qkv[h3 * D:(h3 + 1) * D],
        b_qkv[C + h2 * D:C + (h2 + 1) * D],
        b_qkv[C + h3 * D:C + (h3 + 1) * D]], 0)               # [512]

    wv_aug = np.zeros((C + 1, VW), np.float32)
    for i, h in enumerate(heads):
        wv_aug[:C, i * (D + 1):i * (D + 1) + D] = v_rows(h).T
        wv_aug[C, i * (D + 1):i * (D + 1) + D] = \
            b_qkv[2 * C + h * D:2 * C + (h + 1) * D]
        wv_aug[C, i * (D + 1) + D] = 1.0

    woa = np.concatenate([w_out[:, h * D:(h + 1) * D].T for h in heads], 0)

    return {
        "xT": xTa,
        "wqk": np.ascontiguousarray(wqk_rows.T.astype(bf)),
        "bq": np.ascontiguousarray(
            bqk.reshape(4, 128).T.astype(np.float32)),
        "wv": np.ascontiguousarray(wv_aug.astype(bf)),
        "wo": np.ascontiguousarray(woa.astype(np.float32)),
    }


def assemble_output(partials, b_out):
    """partials: list of 8 [N, C] arrays (core order). Returns [B, N, C]."""
    y = np.empty((B, N, C), np.float32)
    for b in range(B):
        acc = partials[HPC * b].astype(np.float32).copy()
        for g in range(1, HPC):
            acc += partials[HPC * b + g]
        y[b] = acc + b_out.astype(np.float32)
    return y


_NC_CACHE = {}


def run(inputs, trace=False):
    """Returns (y_full [B,N,C] f32, exec_time_ns or None)."""
    x = np.asarray(inputs["x"], np.float32)
    w_qkv = np.asarray(inputs["w_qkv"], np.float32)
    b_qkv = np.asarray(inputs["b_qkv"], np.float32)
    w_out = np.asarray(inputs["w_out"], np.float32)
    b_out = np.asarray(inputs["b_out"], np.float32)

    if "nc" not in _NC_CACHE:
        _NC_CACHE["nc"] = build_bass()
    nc = _NC_CACHE["nc"]

    in_maps = [prep_core_inputs(x, w_qkv, b_qkv, w_out, core)
               for core in range(N_CORES)]
    res = run_bass_kernel_spmd(nc, in_maps, list(range(N_CORES)),
                               trace=trace)
    partials = [res.results[i]["y"] for i in range(N_CORES)]
    return assemble_output(partials, b_out), res.exec_time_ns


def kernel(**inputs):
    y, _ = run(inputs, trace=False)
    return y
